# revision 24
# baseline (speedup 1.0000x reference)
"""PointNet++ encoder (nn_PointNetEncoder) as a Bass/Tile kernel for Trainium2.

Sharding: data-parallel over the B=8 cloud axis, one cloud per NeuronCore.
Each core runs the full per-cloud pipeline on device:
  FPS(2048->1024) -> radius(r=.2,K=64) -> PointNetConv MLP[3,64,64,128] max-agg
  FPS(1024->256)  -> radius(r=.4,K=64) -> PointNetConv MLP[131,128,128,256] max-agg
  MLP[259,256,512,1024] -> global max pool -> [1024]

Host side only shards inputs / stacks outputs and feeds a few constant
index tensors; all math runs on the NeuronCores.
"""

import os
import sys

for _p in ("/opt/trn_rl_repo", "/root/.axon_site/_ro/trn_rl_repo"):
    if os.path.isdir(_p) and _p not in sys.path:
        sys.path.insert(0, _p)

import numpy as np

import concourse.bacc as bacc
import concourse.bass as bass
import concourse.mybir as mybir
from concourse.tile import TileContext
from concourse.bass_utils import run_bass_kernel_spmd

import concourse.dve_ops as _dve_ops
from concourse.dve_ops import DveOp as _DveOp
from concourse.dve_spec import Spec as _Spec, Src0 as _Src0, Src1 as _Src1, sq as _sq

if "SQDIFF_ANT" not in _dve_ops._SUB_OPCODE_FOR_NAME:
    _SQDIFF = _DveOp(
        "SQDIFF_ANT",
        _Spec(body=_sq(_Src0 - _Src1),
              reference=lambda in0, in1, s0, s1, imm2:
                  np.square(in0.astype(np.float32)
                            - np.asarray(in1, np.float32).reshape(in0.shape))),
        subdim=False,
        uops_sha={"v3": "eed49934a849c087"},
    )
    _dve_ops.OPS.append(_SQDIFF)
    _dve_ops.CUSTOM_DVE_SPECS[_SQDIFF.name] = _SQDIFF.spec
    _dve_ops._SUB_OPCODE_FOR_NAME[_SQDIFF.name] =         _dve_ops._CUSTOM_DVE_ROW_BASE + len(_dve_ops.OPS) - 1
else:
    _SQDIFF = next(o for o in _dve_ops.OPS if o.name == "SQDIFF_ANT")

F32 = mybir.dt.float32
U32 = mybir.dt.uint32
I16 = mybir.dt.int16
AX = mybir.AxisListType
OP = mybir.AluOpType
ACTF = mybir.ActivationFunctionType

B, N, K = 8, 2048, 64
R1, R2 = 0.2, 0.4
S1, S2 = N // 2, N // 8          # 1024, 256
NEG = -1.0e30


def _ceil(a, b):
    return (a + b - 1) // b


# ---------------------------------------------------------------------------
# device kernel builder
# ---------------------------------------------------------------------------

def build(n_iter1=S1, n_iter2=S2, fps_unroll=8, dump=(), fps_mode="mask"):
    nc = bacc.Bacc("TRN2", target_bir_lowering=False, debug=False,
                   enable_asserts=False, num_devices=8)

    dt = F32

    def din(name, shape):
        return nc.dram_tensor(name, list(shape), F32, kind="ExternalInput").ap()

    pos_d = din("pos", (N, 3))
    W1a = din("W1a", (3, 64));    b1a = din("b1a", (64,))
    W1b = din("W1b", (64, 64));   b1b = din("b1b", (64,))
    W1c = din("W1c", (64, 128));  b1c = din("b1c", (128,))
    W2a = din("W2a", (131, 128)); b2a = din("b2a", (128,))
    W2b = din("W2b", (128, 128)); b2b = din("b2b", (128,))
    W2c = din("W2c", (128, 256)); b2c = din("b2c", (256,))
    W3a = din("W3a", (259, 256)); b3a = din("b3a", (256,))
    W3b = din("W3b", (256, 512)); b3b = din("b3b", (512,))
    W3c = din("W3c", (512, 1024)); b3c = din("b3c", (1024,))
    iotaF1 = nc.dram_tensor("iotaF1", [32, 64], F32, kind="ExternalInput").ap()
    iotaF2 = nc.dram_tensor("iotaF2", [32, 32], F32, kind="ExternalInput").ap()
    iotaRow = nc.dram_tensor("iotaRow", [1, 32], F32, kind="ExternalInput").ap()
    selI3 = nc.dram_tensor("selI3", [3, 5], F32, kind="ExternalInput").ap()
    selSQ = nc.dram_tensor("selSQ", [3, 5], F32, kind="ExternalInput").ap()
    selONE = nc.dram_tensor("selONE", [1, 5], F32, kind="ExternalInput").ap()
    sel2I = nc.dram_tensor("sel2I", [3, 5], F32, kind="ExternalInput").ap()
    selNSQ = nc.dram_tensor("selNSQ", [3, 5], F32, kind="ExternalInput").ap()
    selR1 = nc.dram_tensor("selR1", [1, 5], F32, kind="ExternalInput").ap()
    selR2 = nc.dram_tensor("selR2", [1, 5], F32, kind="ExternalInput").ap()
    ident128 = nc.dram_tensor("ident128", [128, 128], F32, kind="ExternalInput").ap()
    rep16 = nc.dram_tensor("rep16", [16, 128], F32, kind="ExternalInput").ap()

    out_d = nc.dram_tensor("out", [1024], F32, kind="ExternalOutput").ap()

    nbr_scratch = nc.dram_tensor("nbr_scratch", [128, K], I16).ap()
    nbr_rep_scr = nc.dram_tensor("nbr_rep_scr", [8, 16, 512], I16).ap()
    cpos_scr = nc.dram_tensor("cpos_scr", [3, S1], F32).ap()

    with TileContext(nc) as tc:
        import contextlib
        with contextlib.ExitStack() as ctx:
            persist = ctx.enter_context(tc.tile_pool(name="persist", bufs=1))
            scratch = ctx.enter_context(tc.tile_pool(name="scratch", bufs=2))
            psum = ctx.enter_context(tc.tile_pool(name="psum", bufs=2, space="PSUM"))

            _dump_reg = {}

            def mm_ps(shape, tag="mm"):
                return psum.tile(shape, dt, tag=tag, name=tag)

            # ---------------- phase A: load + layouts ----------------
            aug_p1 = persist.tile([5, N], dt, tag="aug_p1", name="aug_p1")
            nc.sync.dma_start(aug_p1[0:3, :], pos_d.rearrange("j c -> c j"))
            pos_cm1 = persist.tile([32, 3, 64], dt, tag="pos_cm1", name="pos_cm1")
            for c in range(3):
                nc.sync.dma_start(pos_cm1[:, c, :],
                                  pos_d[:, c].rearrange("(p f) -> p f", p=32, f=64))

            io_F1 = persist.tile([32, 64], F32, tag="io_F1", name="io_F1")
            nc.sync.dma_start(io_F1[:], iotaF1)
            io_F2 = persist.tile([32, 32], F32, tag="io_F2", name="io_F2")
            nc.sync.dma_start(io_F2[:], iotaF2)
            io_R = persist.tile([1, 32], F32, tag="io_R", name="io_R")
            nc.sync.dma_start(io_R[:], iotaRow)

            def wtile(ap_, shape, tag):
                t = persist.tile(list(shape), dt, tag=tag, name=tag)
                nc.sync.dma_start(t[:], ap_)
                return t

            sW1a = wtile(W1a, (3, 64), "sW1a")
            sW1b = wtile(W1b, (64, 64), "sW1b")
            sW1c = wtile(W1c, (64, 128), "sW1c")
            sW2ax = wtile(W2a[0:128, :], (128, 128), "sW2ax")
            sW2ap = wtile(W2a[128:131, :], (3, 128), "sW2ap")
            sW2b = wtile(W2b, (128, 128), "sW2b")
            sW2c = wtile(W2c, (128, 256), "sW2c")
            sW3a_h = [wtile(W3a[i * 128:(i + 1) * 128, :], (128, 256), f"sW3a{i}")
                      for i in range(2)]
            sW3a_p = wtile(W3a[256:259, :], (3, 256), "sW3ap")
            sW3b = [wtile(W3b[i * 128:(i + 1) * 128, :], (128, 512), f"sW3b{i}")
                    for i in range(2)]
            sW3c = [wtile(W3c[i * 128:(i + 1) * 128, :], (128, 1024), f"sW3c{i}")
                    for i in range(4)]

            def btile(ap_, m, tag):
                p = min(m, 128)
                t = persist.tile([p, _ceil(m, 128)], dt, tag=tag, name=tag)
                nc.sync.dma_start(t[:], ap_.rearrange("(g p) -> p g", p=p))
                return t

            sb1a = btile(b1a, 64, "sb1a"); sb1b = btile(b1b, 64, "sb1b")
            sb1c = btile(b1c, 128, "sb1c")
            sb2a = btile(b2a, 128, "sb2a"); sb2b = btile(b2b, 128, "sb2b")
            sb2c = btile(b2c, 256, "sb2c")
            sb3a = btile(b3a, 256, "sb3a"); sb3b = btile(b3b, 512, "sb3b")
            sb3c = btile(b3c, 1024, "sb3c")

            sSelI3 = wtile(selI3, (3, 5), "sSelI3")
            sSelSQ = wtile(selSQ, (3, 5), "sSelSQ")
            sSelONE = wtile(selONE, (1, 5), "sSelONE")
            sSel2I = wtile(sel2I, (3, 5), "sSel2I")
            sSelNSQ = wtile(selNSQ, (3, 5), "sSelNSQ")
            sSelR = {0.2: wtile(selR1, (1, 5), "sSelR1"),
                     0.4: wtile(selR2, (1, 5), "sSelR2")}
            onesrow = persist.tile([1, 512], dt, tag="onesrow", name="onesrow")
            nc.vector.memset(onesrow[:], 1.0)
            sIdent = wtile(ident128, (128, 128), "sIdent")
            sRep16 = wtile(rep16, (16, 128), "sRep16")

            def build_aug_p(aug_p, src3, npts):
                """aug_p[5, npts] = rows (x, y, z, |p|^2, 1) from src3 [3, npts]."""
                sqs = scratch.tile([3, npts], dt, tag="sqs", name="sqs")
                nc.vector.tensor_tensor(sqs[:], src3, src3, OP.mult)
                for nb in range(npts // 512):
                    sl = slice(nb * 512, (nb + 1) * 512)
                    ps = psum.tile([5, 512], dt, tag="aug", name="aug")
                    nc.tensor.matmul(out=ps[:], lhsT=sSelI3[:], rhs=src3[:, sl],
                                     start=True, stop=False)
                    nc.tensor.matmul(out=ps[:], lhsT=sSelSQ[:], rhs=sqs[:, sl],
                                     start=False, stop=False)
                    nc.tensor.matmul(out=ps[:], lhsT=sSelONE[:], rhs=onesrow[:],
                                     start=False, stop=True)
                    nc.vector.tensor_copy(aug_p[:, sl], ps[:])

            build_aug_p(aug_p1, aug_p1[0:3, :], N)

            # ---------------- FPS (shared routine) ----------------
            def fps(npts, n_iter, pos_cm, posT3, io_F, cposT3):
                Fdim = npts // 32
                sfx = f"_{npts}"

                def ptile(name, shape, dtype=dt):
                    return persist.tile(shape, dtype, tag=name + sfx, name=name + sfx)

                d = ptile("fps_d", [32, Fdim])
                prep = ptile("prep", [32, 32])
                pb = ptile("pb", [32, 32])
                u = ptile("u", [32, 3, Fdim])
                dc = ptile("dc", [32, Fdim])
                m8 = ptile("m8", [32, 32])
                F8 = ptile("F8", [32, 32], U32)
                F8f = ptile("F8f", [32, 1])
                mT = ptile("mT", [32, 32])
                g8 = ptile("g8", [1, 8])
                P8 = ptile("P8", [1, 8], U32)
                P8f = ptile("P8f", [1, 1])
                rsel = ptile("rsel", [32, 32])
                gcol = ptile("gcol", [32, 32])
                oh = ptile("oh", [32, Fdim])
                moh = ptile("moh", [32, 3, Fdim])
                s3 = ptile("s3", [32, 32])
                s3T = ptile("s3T", [32, 32])
                pxyz = ptile("pxyz", [3, 1])
                for t in (prep, m8, rsel, s3):
                    nc.vector.memset(t[:], 0.0)

                def bcast_pb(pxyz31):
                    nc.vector.tensor_copy(prep[0:3, :], pxyz31.to_broadcast([3, 32]))
                    nc.vector.transpose(pb[:], prep[:])

                def dist_update(first):
                    nc.vector._custom_dve(
                        _SQDIFF, out=u[:], in0=pos_cm[:],
                        in1=pb[:, 0:3].to_broadcast([32, 3, Fdim]))
                    nc.vector.tensor_reduce(
                        dc[:], u[:].rearrange("p c f -> p f c"), AX.X, OP.add)
                    if first:
                        nc.vector.tensor_copy(d[:], dc[:])
                    else:
                        nc.vector.tensor_tensor(d[:], d[:], dc[:], OP.min)

                nc.vector.tensor_copy(cposT3[:, 0:1], posT3[:, 0:1])
                bcast_pb(posT3[:, 0:1])
                dist_update(first=True)

                FT = ptile("FT", [32, 32], U32)
                nc.vector.memset(FT[:], 0)

                def body_reg(iv):
                    nc.vector.max(m8[:, 0:8], d[:])
                    nc.vector.max_index(F8[:, 0:8], m8[:, 0:8], d[:])
                    nc.vector.transpose(mT[:], m8[:])
                    nc.vector.transpose(FT[:], F8[:])
                    nc.vector.max(g8[:], mT[0:1, :])
                    nc.vector.max_index(P8[:], g8[:], mT[0:1, :])
                    rP = nc.vector.value_load(P8[0:1, 0:1], min_val=0, max_val=31)
                    rF = nc.vector.value_load(FT[0:1, bass.ds(rP, 1)],
                                              min_val=0, max_val=Fdim - 1)
                    rJ = rP * Fdim + rF
                    nc.vector.tensor_copy(pxyz[:], posT3[:, bass.ds(rJ, 1)])
                    if isinstance(iv, int):
                        dst = cposT3[:, iv:iv + 1]
                    else:
                        dst = cposT3[:, bass.ds(iv, 1)]
                    nc.vector.tensor_copy(dst, pxyz[:])
                    bcast_pb(pxyz[:])
                    dist_update(first=False)

                def body(iv):
                    nc.vector.max(m8[:, 0:8], d[:])
                    nc.vector.max_index(F8[:, 0:8], m8[:, 0:8], d[:])
                    nc.vector.transpose(mT[:], m8[:])
                    nc.vector.max(g8[:], mT[0:1, :])
                    nc.vector.max_index(P8[:], g8[:], mT[0:1, :])
                    nc.vector.tensor_copy(P8f[:], P8[:, 0:1])
                    nc.vector.tensor_scalar(rsel[0:1, :], io_R[:], P8f[:], None,
                                            op0=OP.is_equal)
                    nc.vector.transpose(gcol[:], rsel[:])
                    nc.vector.tensor_copy(F8f[:], F8[:, 0:1])
                    nc.vector.tensor_scalar(oh[:], io_F[:], F8f[:], gcol[:, 0:1],
                                            op0=OP.is_equal, op1=OP.mult)
                    for c in range(3):
                        nc.vector.scalar_tensor_tensor(
                            moh[:, c, :], pos_cm[:, c, :], 1.0, oh[:],
                            op0=OP.mult, op1=OP.mult,
                            accum_out=s3[:, c:c + 1])
                    nc.vector.transpose(s3T[:], s3[:])
                    nc.vector.tensor_reduce(pxyz[:], s3T[0:3, :], AX.X, OP.add)
                    if isinstance(iv, int):
                        dst = cposT3[:, iv:iv + 1]
                    else:
                        dst = cposT3[:, bass.ds(iv, 1)]
                    nc.vector.tensor_copy(dst, pxyz[:])
                    bcast_pb(pxyz[:])
                    dist_update(first=False)

                bodyfn = body_reg if fps_mode == "reg" else body
                if n_iter > 1:
                    if fps_unroll <= 1:
                        for i in range(1, n_iter):
                            bodyfn(i)
                    else:
                        def unrollable_body(iv0, unroll):
                            for i in range(unroll):
                                bodyfn(iv0 + i)
                        tc.For_i_unrolled_general(
                            1, n_iter, 1, unrollable_body,
                            max_unroll=fps_unroll,
                            hint_engines=(mybir.EngineType.DVE,))

            aug_p2 = persist.tile([5, S1], dt, tag="aug_p2", name="aug_p2")
            nc.vector.memset(aug_p2[:], 0.0)
            cposT3 = aug_p2[0:3, :]
            fps(N, n_iter1, pos_cm1, aug_p1[0:3, :], io_F1, cposT3)

            # ---------------- radius + top-K (shared) ----------------
            def augment_centroids(cposT, s_total, r, tag):
                """aug_c [5, s_total] = rows (2cx, 2cy, 2cz, -1, r^2-|c|^2)."""
                aug_c = persist.tile([5, s_total], dt, tag=tag, name=tag)
                csq = scratch.tile([3, s_total], dt, tag="sqs", name="sqs")
                nc.vector.tensor_tensor(csq[:], cposT, cposT, OP.mult)
                for nb in range(_ceil(s_total, 512)):
                    w = min(512, s_total - nb * 512)
                    sl = slice(nb * 512, nb * 512 + w)
                    ps = psum.tile([5, 512], dt, tag="aug", name="aug")
                    nc.tensor.matmul(out=ps[:, 0:w], lhsT=sSel2I[:], rhs=cposT[:, sl],
                                     start=True, stop=False)
                    nc.tensor.matmul(out=ps[:, 0:w], lhsT=sSelNSQ[:], rhs=csq[:, sl],
                                     start=False, stop=False)
                    nc.tensor.matmul(out=ps[:, 0:w], lhsT=sSelR[r][:], rhs=onesrow[:, 0:w],
                                     start=False, stop=True)
                    nc.vector.tensor_copy(aug_c[:, sl], ps[:, 0:w])
                return aug_c

            def radius_chunk(aug_c_sl, aug_p, npts, nbr16_rep, nrep):
                score = scratch.tile([128, npts], dt, tag=f"score{npts}", name=f"score{npts}")
                for nb in range(npts // 512):
                    sl = slice(nb * 512, (nb + 1) * 512)
                    ps = mm_ps([128, 512])
                    nc.tensor.matmul(out=ps[:], lhsT=aug_c_sl, rhs=aug_p[:, sl],
                                     start=True, stop=True)
                    nc.vector.tensor_copy(score[:, sl], ps[:])
                val64 = scratch.tile([128, K], dt, tag="val64", name="val64")
                nbr = scratch.tile([128, K], U32, tag="nbr", name="nbr")
                for r in range(K // 8):
                    sl = slice(r * 8, r * 8 + 8)
                    nc.vector.max(val64[:, sl], score[:])
                    nc.vector.max_index(nbr[:, sl], val64[:, sl], score[:])
                    if r < K // 8 - 1:
                        nc.vector.match_replace(score[:], val64[:, sl], score[:], NEG)
                inval = scratch.tile([128, K], U32, tag="inval", name="inval")
                nc.vector.tensor_scalar(inval[:], val64[:], 0.0, None, op0=OP.is_lt)
                nc.vector.copy_predicated(nbr[:], inval[:],
                                          nbr[:, 0:1].to_broadcast([128, K]))
                if "nbr0" in dump and not _dump_reg.get("_nbr0done"):
                    _dump_reg["_nbr0done"] = True
                    _d1 = nc.dram_tensor("dbg_nbr0", [128, K], U32,
                                         kind="ExternalOutput").ap()
                    nc.sync.dma_start(_d1, nbr[:])
                    _d2 = nc.dram_tensor("dbg_val0", [128, K], F32,
                                         kind="ExternalOutput").ap()
                    nc.sync.dma_start(_d2, val64[:])
                nbr_f = scratch.tile([128, K], dt, tag="nbr_f", name="nbr_f")
                nc.vector.tensor_copy(nbr_f[:], nbr[:])
                # wrapped edge-list layout [16, 512]: w16f[p, s*4+q] = nbr[s, q*16+p]
                w16f = scratch.tile([16, 512], dt, tag="w16f", name="w16f")
                for q in range(4):
                    pst = psum.tile([16, 128], dt, tag="tr16", name="tr16")
                    nc.tensor.transpose(pst[:], nbr_f[:, q * 16:(q + 1) * 16],
                                        sIdent[:])
                    nc.vector.tensor_copy(
                        w16f[:].rearrange("p (s q) -> p s q", q=4)[:, :, q],
                        pst[:])
                # replicate to all cores' partition groups, cast to i16
                cfeat = nbr16_rep.shape[0]
                psr = psum.tile([cfeat, 512], dt, tag="mm", name="mm")
                nc.tensor.matmul(out=psr[:], lhsT=sRep16[:, 0:cfeat], rhs=w16f[:],
                                 start=True, stop=True)
                nc.vector.tensor_copy(nbr16_rep[:], psr[:])

            # ---------------- conv (shared) ----------------
            def conv_chunk(AT, npts, cfeat, nbr16_rep, Wb_, bb_, Wc_, bias_a,
                           BcT, col0, hagg_list, hagg_col0):
                # stream the 8192 edges of this 128-centroid chunk in 4
                # slices of 2048 edges (32 centroids each)
                for sli in range(4):
                    esl = slice(sli * 2048, (sli + 1) * 2048)
                    isl = slice(sli * 128, (sli + 1) * 128)
                    csl = slice(col0 + sli * 32, col0 + (sli + 1) * 32)
                    GT = scratch.tile([cfeat, 2048], dt, tag="GT", name="GT")
                    nc.gpsimd.ap_gather(GT[:], AT, nbr16_rep[:, isl],
                                        channels=cfeat, num_elems=npts, d=1,
                                        num_idxs=2048)
                    if "GT0" in dump and not _dump_reg.get("_gt0done"):
                        _dump_reg["_gt0done"] = True
                        _d3 = nc.dram_tensor("dbg_GT0", [cfeat, 2048], F32,
                                             kind="ExternalOutput").ap()
                        nc.sync.dma_start(_d3, GT[:])
                        _d4 = nc.dram_tensor("dbg_idx0", [cfeat, 128], I16,
                                             kind="ExternalOutput").ap()
                        nc.sync.dma_start(_d4, nbr16_rep[:, isl])
                    h1 = scratch.tile([cfeat, 2048], dt, tag="h1c", name="h1c")
                    nc.vector.scalar_tensor_tensor(
                        h1[:].rearrange("p (s k) -> p s k", k=K),
                        GT[:].rearrange("p (s k) -> p s k", k=K),
                        bias_a[:, 0:1],
                        BcT[:, csl].rearrange("p (s a) -> p s a", a=1)
                        .to_broadcast([cfeat, 32, K]),
                        op0=OP.add, op1=OP.subtract)
                    nc.scalar.activation(h1[:], h1[:], ACTF.Relu)
                    h2 = scratch.tile([cfeat, 2048], dt, tag="h2c", name="h2c")
                    for nb in range(4):
                        sl = slice(nb * 512, (nb + 1) * 512)
                        ps = mm_ps([cfeat, 512])
                        nc.tensor.matmul(out=ps[:], lhsT=Wb_[:], rhs=h1[:, sl],
                                         start=True, stop=True)
                        nc.scalar.activation(h2[:, sl], ps[:], ACTF.Relu,
                                             bias=bb_[:, 0:1])
                    Mout = Wc_.shape[1]
                    for mg in range(_ceil(Mout, 128)):
                        for nb in range(4):
                            sl = slice(nb * 512, (nb + 1) * 512)
                            ps = mm_ps([128, 512])
                            nc.tensor.matmul(out=ps[:],
                                             lhsT=Wc_[:, mg * 128:(mg + 1) * 128],
                                             rhs=h2[:, sl], start=True, stop=True)
                            c0 = hagg_col0 + sli * 32 + nb * 8
                            nc.vector.tensor_reduce(
                                hagg_list[mg][:, c0:c0 + 8],
                                ps[:].rearrange("p (s k) -> p s k", k=K),
                                AX.X, OP.max)

            # ============ SA1 ============
            aug_c1 = augment_centroids(cposT3, S1, R1, "aug_c1")
            A1T = persist.tile([64, N], dt, tag="A1T", name="A1T")
            for nb in range(N // 512):
                sl = slice(nb * 512, (nb + 1) * 512)
                ps = mm_ps([64, 512])
                nc.tensor.matmul(out=ps[:], lhsT=sW1a[:], rhs=aug_p1[0:3, sl],
                                 start=True, stop=True)
                nc.vector.tensor_copy(A1T[:, sl], ps[:])
            Bc1 = persist.tile([64, S1], dt, tag="Bc1", name="Bc1")
            for nb in range(S1 // 512):
                sl = slice(nb * 512, (nb + 1) * 512)
                ps = mm_ps([64, 512])
                nc.tensor.matmul(out=ps[:], lhsT=sW1a[:], rhs=cposT3[:, sl],
                                 start=True, stop=True)
                nc.vector.tensor_copy(Bc1[:, sl], ps[:])

            h1_all = persist.tile([128, S1], dt, tag="h1_all", name="h1_all")
            for ch in range(S1 // 128):
                nbr16_rep = scratch.tile([64, 512], I16, tag="nbrrep1", name="nbrrep1")
                radius_chunk(aug_c1[:, ch * 128:(ch + 1) * 128], aug_p1, N,
                             nbr16_rep, 4)
                conv_chunk(A1T[:], N, 64, nbr16_rep, sW1b, sb1b, sW1c, sb1a,
                           Bc1, ch * 128, [h1_all], ch * 128)
            nc.vector.tensor_scalar(h1_all[:], h1_all[:], sb1c[:, 0:1], None, op0=OP.add)

            # ============ FPS2 over p1 = cposT3 ============
            pos_cm2 = persist.tile([32, 3, 32], dt, tag="pos_cm2", name="pos_cm2")
            nc.sync.dma_start(cpos_scr, cposT3)
            for c in range(3):
                nc.sync.dma_start(pos_cm2[:, c, :],
                                  cpos_scr[c].rearrange("(p f) -> p f", p=32, f=32))
            c2pos = persist.tile([3, S2], dt, tag="c2pos", name="c2pos")
            nc.vector.memset(c2pos[:], 0.0)
            fps(S1, n_iter2, pos_cm2, cposT3, io_F2, c2pos[:])

            build_aug_p(aug_p2, cposT3, S1)

            # ============ SA2 ============
            aug_c2 = augment_centroids(c2pos[:], S2, R2, "aug_c2")
            A2T = persist.tile([128, S1], dt, tag="A2T", name="A2T")
            for nb in range(S1 // 512):
                sl = slice(nb * 512, (nb + 1) * 512)
                ps = mm_ps([128, 512])
                nc.tensor.matmul(out=ps[:], lhsT=sW2ax[:], rhs=h1_all[:, sl],
                                 start=True, stop=False)
                nc.tensor.matmul(out=ps[:], lhsT=sW2ap[:], rhs=cposT3[:, sl],
                                 start=False, stop=True)
                nc.vector.tensor_copy(A2T[:, sl], ps[:])
            Bc2 = persist.tile([128, S2], dt, tag="Bc2", name="Bc2")
            ps = mm_ps([128, 512])
            nc.tensor.matmul(out=ps[:, 0:S2], lhsT=sW2ap[:], rhs=c2pos[:],
                             start=True, stop=True)
            nc.vector.tensor_copy(Bc2[:], ps[:, 0:S2])

            h2_all = [persist.tile([128, S2], dt, tag=f"h2_all{i}", name=f"h2_all{i}") for i in range(2)]
            for ch in range(S2 // 128):
                nbr16_rep = scratch.tile([128, 512], I16, tag="nbrrep2", name="nbrrep2")
                radius_chunk(aug_c2[:, ch * 128:(ch + 1) * 128], aug_p2, S1,
                             nbr16_rep, 8)
                conv_chunk(A2T[:], S1, 128, nbr16_rep, sW2b, sb2b, sW2c, sb2a,
                           Bc2, ch * 128, h2_all, ch * 128)
            for mg in range(2):
                nc.vector.tensor_scalar(h2_all[mg][:], h2_all[mg][:],
                                        sb2c[:, mg:mg + 1], None, op0=OP.add)

            # ============ MLP3 + global max ============
            X1 = [scratch.tile([128, S2], dt, tag=f"X1_{i}", name=f"X1_{i}") for i in range(2)]
            for mg in range(2):
                msl = slice(mg * 128, (mg + 1) * 128)
                ps = mm_ps([128, 512])
                nc.tensor.matmul(out=ps[:, 0:S2], lhsT=sW3a_h[0][:, msl],
                                 rhs=h2_all[0][:], start=True, stop=False)
                nc.tensor.matmul(out=ps[:, 0:S2], lhsT=sW3a_h[1][:, msl],
                                 rhs=h2_all[1][:], start=False, stop=False)
                nc.tensor.matmul(out=ps[:, 0:S2], lhsT=sW3a_p[:, msl],
                                 rhs=c2pos[:], start=False, stop=True)
                nc.scalar.activation(X1[mg][:], ps[:, 0:S2], ACTF.Relu,
                                     bias=sb3a[:, mg:mg + 1])
            X2 = [scratch.tile([128, S2], dt, tag=f"X2_{i}", name=f"X2_{i}") for i in range(4)]
            for mg in range(4):
                msl = slice(mg * 128, (mg + 1) * 128)
                ps = mm_ps([128, 512])
                for kg in range(2):
                    nc.tensor.matmul(out=ps[:, 0:S2], lhsT=sW3b[kg][:, msl],
                                     rhs=X1[kg][:], start=(kg == 0), stop=(kg == 1))
                nc.scalar.activation(X2[mg][:], ps[:, 0:S2], ACTF.Relu,
                                     bias=sb3b[:, mg:mg + 1])
            _dump_reg.update(dict(
                cposT3=cposT3, A1T=A1T[:], Bc1=Bc1[:], h1_all=h1_all[:],
                c2pos=c2pos[:], A2T=A2T[:], aug_p1=aug_p1[:], aug_c1=aug_c1[:],
                h2_all0=h2_all[0][:], h2_all1=h2_all[1][:]))
            gout = scratch.tile([128, 8], dt, tag="gout", name="gout")
            for mg in range(8):
                msl = slice(mg * 128, (mg + 1) * 128)
                ps = mm_ps([128, 512])
                for kg in range(4):
                    nc.tensor.matmul(out=ps[:, 0:S2], lhsT=sW3c[kg][:, msl],
                                     rhs=X2[kg][:], start=(kg == 0), stop=(kg == 3))
                nc.vector.tensor_reduce(gout[:, mg:mg + 1], ps[:, 0:S2], AX.X, OP.max)
            nc.vector.tensor_tensor(gout[:], gout[:], sb3c[:], OP.add)
            nc.sync.dma_start(out_d.rearrange("(g p) -> p g", p=128), gout[:])
            for _dn in dump:
                if _dn not in _dump_reg:
                    continue
                _ap = _dump_reg[_dn]
                _dd = nc.dram_tensor(f"dbg_{_dn}", list(_ap.shape), _ap.dtype,
                                     kind="ExternalOutput").ap()
                nc.sync.dma_start(_dd, _ap)

    nc.compile()
    return nc


# ---------------------------------------------------------------------------
# host wrapper
# ---------------------------------------------------------------------------

_NC_CACHE = {}


def _get_nc():
    if "nc" not in _NC_CACHE:
        _NC_CACHE["nc"] = build()
    return _NC_CACHE["nc"]


def _aux_inputs():
    selI3 = np.zeros((3, 5), np.float32); selI3[:, 0:3] = np.eye(3)
    selSQ = np.zeros((3, 5), np.float32); selSQ[:, 3] = 1.0
    selONE = np.zeros((1, 5), np.float32); selONE[0, 4] = 1.0
    sel2I = 2.0 * selI3
    selNSQ = np.zeros((3, 5), np.float32); selNSQ[:, 4] = -1.0
    selR1 = np.zeros((1, 5), np.float32); selR1[0, 3] = -1.0; selR1[0, 4] = R1 * R1
    selR2 = np.zeros((1, 5), np.float32); selR2[0, 3] = -1.0; selR2[0, 4] = R2 * R2
    return {
        "iotaF1": np.broadcast_to(np.arange(64, dtype=np.float32), (32, 64)).copy(),
        "iotaF2": np.broadcast_to(np.arange(32, dtype=np.float32), (32, 32)).copy(),
        "iotaRow": np.arange(32, dtype=np.float32).reshape(1, 32).copy(),
        "selI3": selI3, "selSQ": selSQ, "selONE": selONE, "sel2I": sel2I,
        "selNSQ": selNSQ, "selR1": selR1, "selR2": selR2,
        "ident128": np.eye(128, dtype=np.float32),
        "rep16": np.equal(np.arange(128)[None, :] % 16,
                          np.arange(16)[:, None]).astype(np.float32),
    }


def kernel(**inputs) -> np.ndarray:
    nc = _get_nc()
    pos = np.asarray(inputs["pos"], dtype=np.float32)
    aux = _aux_inputs()
    weights = {k: np.ascontiguousarray(np.asarray(v, dtype=np.float32))
               for k, v in inputs.items() if k != "pos"}
    in_maps = []
    for b in range(B):
        m = {"pos": np.ascontiguousarray(pos[b])}
        m.update(weights)
        m.update(aux)
        in_maps.append(m)
    res = run_bass_kernel_spmd(nc, in_maps, list(range(B)))
    return np.stack([res.results[b]["out"] for b in range(B)], axis=0)


if __name__ == "__main__":
    nc = build(n_iter1=4, n_iter2=3, fps_unroll=1)
    print("build OK")
